# revision 4
# baseline (speedup 1.0000x reference)
"""Trainium2 Bass kernel for the nn_Jastrow problem.

Computes, for x [B=16384, N=16, D=2]:
  - one-body branch: MLP3(concat(x, |x|^2)) averaged over the 16 sites
  - two-body branch: MLP3(6 radial features of r_ij) averaged over 120 pairs
  - readout MLP + exact cusp term  sum_pairs r*exp(-r)

Sharding: pure data-parallel over batch across 8 NeuronCores (2048
samples/core), weights replicated.  All shapes are hardcoded.

Dataflow per core:
  1. front end in [sample-partition, free] layout: pair diffs, r, radial
     features (ACT Ln/Exp only -- r computed as exp(0.5 ln(r2+eps)) to stay
     in one activation-table set), cusp map r*exp(-r) in fp32.
  2. PE transposes features into [unit*feat, sample] layout.
  3. MLPs with block-diagonal weights packing 2 units per matmul; the
     mean over units is folded into the last (linear) layer via PSUM
     accumulation.  gelu on ACT over [128,1024] tiles.
  4. readout; cusp summed over pairs by a ones-vector matmul accumulated
     into the readout PSUM.
"""

import math
import os
import sys

sys.path.insert(0, "/opt/trn_rl_repo")

import numpy as np
from ml_dtypes import bfloat16

import concourse.bacc as bacc
import concourse.bass as bass
import concourse.mybir as mybir
import concourse.tile as tile
from concourse import bass_utils
from concourse._compat import get_trn_type

F32 = mybir.dt.float32
BF16 = mybir.dt.bfloat16
AF = mybir.ActivationFunctionType

B, N, D = 16384, 16, 2
HID, DL = 64, 5
NCORES = 8
BC = B // NCORES            # samples per core = 2048
P = N * (N - 1) // 2        # 120 pairs
# pair order: grouped by offset k: (i, i+k) for k=1..15, i=0..15-k
PAIRS = [(i, i + k) for k in range(1, N) for i in range(N - k)]
NCHUNK = 6                  # feature-transpose chunks of 20 pairs
CPP = P // NCHUNK           # 20 pairs per chunk
NQ = CPP // 2               # 10 two-pair groups per chunk
EPS = 1e-12


# --------------------------------------------------------------------------
# host-side weight packing
# --------------------------------------------------------------------------
def _pack_weights(iw):
    w = {}
    f32 = np.float32

    # psi L1: 10 stationary variants q (two-pair blocks), K=120, M=128 each,
    # laid side by side -> [120, 1280], bf16
    w0 = iw["psi_w0"]                      # [64, 6]
    w0b = np.zeros((P, NQ * 128), f32)
    for q in range(NQ):
        for half in range(2):
            r0 = 12 * q + 6 * half
            c0 = 128 * q + 64 * half
            w0b[r0:r0 + 6, c0:c0 + 64] = w0.T
    w["w0b_psi"] = w0b.astype(bfloat16)

    w1 = iw["psi_w1"]                      # [64, 64]
    w1d = np.zeros((128, 128), f32)
    w1d[:64, :64] = w1.T
    w1d[64:, 64:] = w1.T
    w["w1d_psi"] = w1d.astype(bfloat16)

    w2 = iw["psi_w2"]                      # [5, 64]
    w["w2s_psi"] = (np.vstack([w2.T, w2.T]) / np.float32(P)).astype(bfloat16)

    pw0 = iw["phi_w0"]                     # [64, 3]
    p0b = np.zeros((48, 8 * 128), f32)
    for q in range(8):
        for half in range(2):
            r0 = 6 * q + 3 * half
            c0 = 128 * q + 64 * half
            p0b[r0:r0 + 3, c0:c0 + 64] = pw0.T
    w["w0b_phi"] = p0b.astype(bfloat16)

    pw1 = iw["phi_w1"]
    p1d = np.zeros((128, 128), f32)
    p1d[:64, :64] = pw1.T
    p1d[64:, 64:] = pw1.T
    w["w1d_phi"] = p1d.astype(bfloat16)

    pw2 = iw["phi_w2"]
    w["w2s_phi"] = (np.vstack([pw2.T, pw2.T]) / np.float32(N)).astype(bfloat16)

    w["b01_psi"] = np.concatenate([iw["psi_b0"], iw["psi_b0"]]).astype(f32).reshape(128, 1)
    w["b11_psi"] = np.concatenate([iw["psi_b1"], iw["psi_b1"]]).astype(f32).reshape(128, 1)
    w["b01_phi"] = np.concatenate([iw["phi_b0"], iw["phi_b0"]]).astype(f32).reshape(128, 1)
    w["b11_phi"] = np.concatenate([iw["phi_b1"], iw["phi_b1"]]).astype(f32).reshape(128, 1)
    w["b2_psi"] = iw["psi_b2"].astype(f32).reshape(DL, 1)
    w["b2_phi"] = iw["phi_b2"].astype(f32).reshape(DL, 1)

    # readout: rho_in = concat(phi_out, psi_out); rho_w0 [64, 10]
    rw0 = iw["rho_w0"]
    w["wr0_phi"] = rw0[:, :DL].T.astype(f32).copy()    # [5, 64]
    w["wr0_psi"] = rw0[:, DL:].T.astype(f32).copy()    # [5, 64]
    w["b0_rho"] = iw["rho_b0"].astype(f32).reshape(HID, 1)
    w["wr1"] = iw["rho_w1"].T.astype(f32).copy()       # [64, 1]

    w["ones_p"] = np.ones((P, 1), f32)
    w["c_eps"] = np.full((128, 1), EPS, f32)
    w["ident_f"] = np.eye(128, dtype=f32)
    w["ident_b"] = np.eye(128, dtype=np.float32).astype(bfloat16)
    return w


WEIGHT_SPECS = [
    ("w0b_psi", (P, NQ * 128), BF16),
    ("w1d_psi", (128, 128), BF16),
    ("w2s_psi", (128, DL), BF16),
    ("w0b_phi", (48, 8 * 128), BF16),
    ("w1d_phi", (128, 128), BF16),
    ("w2s_phi", (128, DL), BF16),
    ("b01_psi", (128, 1), F32),
    ("b11_psi", (128, 1), F32),
    ("b01_phi", (128, 1), F32),
    ("b11_phi", (128, 1), F32),
    ("b2_psi", (DL, 1), F32),
    ("b2_phi", (DL, 1), F32),
    ("wr0_phi", (DL, HID), F32),
    ("wr0_psi", (DL, HID), F32),
    ("b0_rho", (HID, 1), F32),
    ("wr1", (HID, 1), F32),
    ("ones_p", (P, 1), F32),
    ("c_eps", (128, 1), F32),
    ("ident_f", (128, 128), F32),
    ("ident_b", (128, 128), BF16),
]


# --------------------------------------------------------------------------
# kernel body
# --------------------------------------------------------------------------
def build_program(bc=BC):
    nsub = bc // 128
    nmega = bc // 512

    nc = bacc.Bacc(get_trn_type() or "TRN2", target_bir_lowering=False,
                   debug=False, num_devices=NCORES)

    x_d = nc.dram_tensor("x", [bc, N, D], F32, kind="ExternalInput")
    y_d = nc.dram_tensor("y", [1, bc], F32, kind="ExternalOutput")
    wd = {name: nc.dram_tensor(name, list(shape), dt, kind="ExternalInput")
          for name, shape, dt in WEIGHT_SPECS}

    off = [0] * (N + 1)
    for k in range(1, N):
        off[k + 1] = off[k] + (N - k)

    with tile.TileContext(nc) as tc:
        with tc.tile_pool(name="persist", bufs=1) as pp:
            # ---- persistent tiles -------------------------------------------
            wt = {}
            for name, shape, dt in WEIGHT_SPECS:
                wt[name] = pp.tile(list(shape), dt, tag=f"w_{name}", name=f"w_{name}")
                nc.sync.dma_start(wt[name][:], wd[name].ap())

            featP = [pp.tile([P, bc], BF16, tag=f"featP{c}", name=f"featP{c}")
                     for c in range(NCHUNK)]
            featF = pp.tile([48, bc], BF16, tag="featF", name="featF")
            mcT = pp.tile([P, bc], F32, tag="mcT", name="mcT")
            rin_psi = [pp.tile([DL, 512], F32, tag=f"rinpsi{m}", name=f"rinpsi{m}")
                       for m in range(nmega)]
            rin_phi = [pp.tile([DL, 512], F32, tag=f"rinphi{m}", name=f"rinphi{m}")
                       for m in range(nmega)]
            fout = pp.tile([1, bc], F32, tag="fout", name="fout")

            # ---- phase 1+2: front end & transposes --------------------------
            with tc.tile_pool(name="front", bufs=1) as pf, \
                 tc.tile_pool(name="psT", bufs=2, space="PSUM") as psT:
                xP = pf.tile([128, nsub * 32], F32, tag="xP", name="xP")
                nc.sync.dma_start(
                    xP[:].rearrange("p (t w) -> p t w", w=32),
                    x_d.ap().rearrange("(t p) n d -> p t (n d)", p=128))
                xv = xP[:].rearrange("p (t w) -> p t w", w=32)

                Fphi = pf.tile([128, nsub * 48], BF16, tag="Fphi", name="Fphi")
                fv = Fphi[:].rearrange("p (t w) -> p t w", w=48)
                nc.vector.tensor_copy(fv[:, :, 0:48:3], xv[:, :, 0:32:2])
                nc.vector.tensor_copy(fv[:, :, 1:48:3], xv[:, :, 1:32:2])
                r2a = pf.tile([128, nsub * 16], F32, tag="r2a", name="r2a")
                r2b = pf.tile([128, nsub * 16], F32, tag="r2b", name="r2b")
                r2av = r2a[:].rearrange("p (t w) -> p t w", w=16)
                r2bv = r2b[:].rearrange("p (t w) -> p t w", w=16)
                nc.vector.tensor_mul(r2av, xv[:, :, 0:32:2], xv[:, :, 0:32:2])
                nc.vector.tensor_mul(r2bv, xv[:, :, 1:32:2], xv[:, :, 1:32:2])
                nc.vector.tensor_add(fv[:, :, 2:48:3], r2av, r2bv)

                drF = pf.tile([128, nsub * 240], F32, tag="drF", name="drF")
                dv = drF[:].rearrange("p (t w) -> p t w", w=240)
                for k in range(1, N):
                    nk = N - k
                    nc.vector.tensor_sub(
                        dv[:, :, 2 * off[k]: 2 * off[k] + 2 * nk],
                        xv[:, :, 0: 2 * nk],
                        xv[:, :, 2 * k: 32])

                dr2 = pf.tile([128, nsub * 240], F32, tag="dr2", name="dr2")
                d2v = dr2[:].rearrange("p (t w) -> p t w", w=240)
                nc.vector.tensor_mul(d2v, dv, dv)
                r2p = pf.tile([128, nsub * P], F32, tag="r2p", name="r2p")
                r2v = r2p[:].rearrange("p (t w) -> p t w", w=P)
                nc.vector.tensor_add(r2v, d2v[:, :, 0:240:2], d2v[:, :, 1:240:2])

                # r = exp(0.5*ln(r2 + eps))  (avoids the sqrt table set)
                lnr2 = pf.tile([128, nsub * P], F32, tag="lnr2", name="lnr2")
                nc.scalar.activation(lnr2[:], r2p[:], AF.Ln, bias=wt["c_eps"][:])
                rT = pf.tile([128, nsub * P], F32, tag="rT", name="rT")
                nc.scalar.activation(rT[:], lnr2[:], AF.Exp, scale=0.5)
                rv = rT[:].rearrange("p (t w) -> p t w", w=P)

                Fpsi = pf.tile([128, nsub * 720], BF16, tag="Fpsi", name="Fpsi")
                pv = Fpsi[:].rearrange("p (t w) -> p t w", w=720)
                nc.scalar.activation(pv[:, :, 0:720:6], rv, AF.Ln, bias=1.0)
                nc.scalar.activation(pv[:, :, 2:720:6], r2v, AF.Exp, scale=-1.0)
                nc.scalar.activation(pv[:, :, 3:720:6], rv, AF.Exp, scale=-0.5)
                e_r = pf.tile([128, nsub * P], F32, tag="e_r", name="e_r")
                nc.scalar.activation(e_r[:], rT[:], AF.Exp, scale=-1.0)
                ev = e_r[:].rearrange("p (t w) -> p t w", w=P)
                nc.vector.tensor_copy(pv[:, :, 4:720:6], ev)
                nc.scalar.activation(pv[:, :, 5:720:6], rv, AF.Exp, scale=-2.0)
                inv1p = pf.tile([128, nsub * P], F32, tag="inv1p", name="inv1p")
                # 1/(1+r) = exp(-ln(1+r)); f0 read back in bf16 (damped path)
                nc.scalar.activation(inv1p[:].rearrange("p (t w) -> p t w", w=P),
                                     pv[:, :, 0:720:6], AF.Exp, scale=-1.0)
                nc.vector.tensor_mul(pv[:, :, 1:720:6], rv,
                                     inv1p[:].rearrange("p (t w) -> p t w", w=P))

                mc = pf.tile([128, nsub * P], F32, tag="mc", name="mc")
                mv = mc[:].rearrange("p (t w) -> p t w", w=P)
                nc.vector.tensor_mul(mv, rv, ev)

                # transposes into [unit, sample] layout
                for m in range(nmega):
                    for c in range(NCHUNK):
                        ps = psT.tile([P, 512], BF16, tag="trB", name="trB")
                        for j in range(4):
                            t = 4 * m + j
                            nc.tensor.transpose(
                                ps[:, 128 * j:128 * (j + 1)],
                                pv[:, t, 120 * c:120 * (c + 1)],
                                wt["ident_b"][:])
                        nc.vector.tensor_copy(featP[c][:, 512 * m:512 * (m + 1)], ps[:])
                    psm = psT.tile([P, 512], F32, tag="trF", name="trF")
                    for j in range(4):
                        t = 4 * m + j
                        nc.tensor.transpose(psm[:, 128 * j:128 * (j + 1)],
                                            mv[:, t, :], wt["ident_f"][:])
                    nc.vector.tensor_copy(mcT[:, 512 * m:512 * (m + 1)], psm[:])
                    psf = psT.tile([48, 512], BF16, tag="trB", name="trB2")
                    for j in range(4):
                        t = 4 * m + j
                        nc.tensor.transpose(psf[:, 128 * j:128 * (j + 1)],
                                            fv[:, t, :], wt["ident_b"][:])
                    nc.vector.tensor_copy(featF[:, 512 * m:512 * (m + 1)], psf[:])

            # ---- phases 3-5: MLPs + tail ------------------------------------
            with tc.tile_pool(name="mlp", bufs=1) as pm, \
                 tc.tile_pool(name="psA", bufs=2, space="PSUM") as psA, \
                 tc.tile_pool(name="psAcc", bufs=1, space="PSUM") as psAcc, \
                 tc.tile_pool(name="psTail", bufs=1, space="PSUM") as psTail:

                GB = 16  # slot depth for gelu-output tiles

                # phi MLP over all megatiles
                for m in range(nmega):
                    cols = slice(512 * m, 512 * (m + 1))
                    g1l = []
                    for jp in range(4):
                        pa = psA.tile([128, 1024], F32, tag="mmA", name="paF")
                        for h in range(2):
                            q = 2 * jp + h
                            nc.tensor.matmul(
                                pa[:, 512 * h:512 * (h + 1)],
                                wt["w0b_phi"][:, 128 * q:128 * (q + 1)],
                                featF[:, cols], start=True, stop=True)
                        g1 = pm.tile([128, 1024], BF16, tag="g1", bufs=GB, name="g1F")
                        nc.scalar.activation(g1[:], pa[:], AF.Gelu, bias=wt["b01_phi"][:])
                        g1l.append(g1)
                    g2l = []
                    for jp in range(4):
                        pb = psA.tile([128, 1024], F32, tag="mmA", name="pbF")
                        for h in range(2):
                            nc.tensor.matmul(
                                pb[:, 512 * h:512 * (h + 1)], wt["w1d_phi"][:],
                                g1l[jp][:, 512 * h:512 * (h + 1)],
                                start=True, stop=True)
                        g2 = pm.tile([128, 1024], BF16, tag="g2", bufs=GB, name="g2F")
                        nc.scalar.activation(g2[:], pb[:], AF.Gelu, bias=wt["b11_phi"][:])
                        g2l.append(g2)
                    acc = psAcc.tile([DL, 512], F32, tag="acc", name="accF")
                    for jp in range(4):
                        for h in range(2):
                            nc.tensor.matmul(
                                acc[:], wt["w2s_phi"][:],
                                g2l[jp][:, 512 * h:512 * (h + 1)],
                                start=(jp == 0 and h == 0),
                                stop=(jp == 3 and h == 1),
                                skip_group_check=True)
                    nc.scalar.activation(rin_phi[m][:], acc[:], AF.Identity,
                                         bias=wt["b2_phi"][:])

                # psi MLP per megatile, two halves of 30 groups
                for m in range(nmega):
                    cols = slice(512 * m, 512 * (m + 1))
                    acc = psAcc.tile([DL, 512], F32, tag="acc", name="accP")
                    for half in range(2):
                        chunks = list(range(3 * half, 3 * half + 3))
                        g1l = []
                        pa = None
                        nslot = 0
                        for q in range(NQ):
                            for c in chunks:
                                if nslot == 0:
                                    pa = psA.tile([128, 1024], F32, tag="mmA", name="paP")
                                nc.tensor.matmul(
                                    pa[:, 512 * nslot:512 * (nslot + 1)],
                                    wt["w0b_psi"][:, 128 * q:128 * (q + 1)],
                                    featP[c][:, cols], start=True, stop=True)
                                if nslot == 1:
                                    g1 = pm.tile([128, 1024], BF16, tag="g1",
                                                 bufs=GB, name="g1P")
                                    nc.scalar.activation(g1[:], pa[:], AF.Gelu,
                                                         bias=wt["b01_psi"][:])
                                    g1l.append(g1)
                                nslot ^= 1
                        g2l = []
                        nslot = 0
                        pb = None
                        for i in range(15):
                            for h in range(2):
                                if nslot == 0:
                                    pb = psA.tile([128, 1024], F32, tag="mmA", name="pbP")
                                nc.tensor.matmul(
                                    pb[:, 512 * nslot:512 * (nslot + 1)],
                                    wt["w1d_psi"][:],
                                    g1l[i][:, 512 * h:512 * (h + 1)],
                                    start=True, stop=True)
                                if nslot == 1:
                                    g2 = pm.tile([128, 1024], BF16, tag="g2",
                                                 bufs=GB, name="g2P")
                                    nc.scalar.activation(g2[:], pb[:], AF.Gelu,
                                                         bias=wt["b11_psi"][:])
                                    g2l.append(g2)
                                nslot ^= 1
                        for i in range(15):
                            for h in range(2):
                                nc.tensor.matmul(
                                    acc[:], wt["w2s_psi"][:],
                                    g2l[i][:, 512 * h:512 * (h + 1)],
                                    start=(half == 0 and i == 0 and h == 0),
                                    stop=(half == 1 and i == 14 and h == 1),
                                    skip_group_check=True)
                    nc.scalar.activation(rin_psi[m][:], acc[:], AF.Identity,
                                         bias=wt["b2_psi"][:])

                    # tail
                    ph = psTail.tile([HID, 512], F32, tag="tail", name="ph")
                    nc.tensor.matmul(ph[:], wt["wr0_phi"][:], rin_phi[m][:],
                                     start=True, stop=False, skip_group_check=True)
                    nc.tensor.matmul(ph[:], wt["wr0_psi"][:], rin_psi[m][:],
                                     start=False, stop=True, skip_group_check=True)
                    hr = pm.tile([HID, 512], F32, tag="hr", bufs=2, name="hr")
                    nc.scalar.activation(hr[:], ph[:], AF.Gelu, bias=wt["b0_rho"][:])
                    pfp = psTail.tile([1, 512], F32, tag="tailf", name="pfp")
                    nc.tensor.matmul(pfp[:], wt["wr1"][:], hr[:],
                                     start=True, stop=False, skip_group_check=True)
                    nc.tensor.matmul(pfp[:], wt["ones_p"][:],
                                     mcT[:, 512 * m:512 * (m + 1)],
                                     start=False, stop=True, skip_group_check=True)
                    nc.scalar.copy(fout[:, 512 * m:512 * (m + 1)], pfp[:])

            nc.sync.dma_start(y_d.ap(), fout[:])

    nc.compile()
    return nc


_prog_cache = {}


def _get_program(bc=BC):
    if bc not in _prog_cache:
        _prog_cache[bc] = build_program(bc)
    return _prog_cache[bc]


def kernel(**inputs):
    x = np.ascontiguousarray(np.asarray(inputs["x"], dtype=np.float32))
    assert x.shape == (B, N, D)
    w = _pack_weights({k: np.asarray(v, dtype=np.float32) for k, v in inputs.items()
                       if k != "x"})
    warr = {}
    for name, shape, dt in WEIGHT_SPECS:
        a = np.ascontiguousarray(w[name])
        assert a.shape == tuple(shape), (name, a.shape, shape)
        warr[name] = a

    nc = _get_program(BC)
    in_maps = [dict(x=x[c * BC:(c + 1) * BC], **warr) for c in range(NCORES)]
    res = bass_utils.run_bass_kernel_spmd(
        nc, in_maps, core_ids=list(range(NCORES)),
        trace=bool(int(os.environ.get("JAS_TRACE", "0"))))
    kernel.last_results = res
    y = np.concatenate([np.asarray(res.results[c]["y"]).reshape(-1)
                        for c in range(NCORES)])
    return y.astype(np.float32)
